# revision 26
# baseline (speedup 1.0000x reference)
"""GAT forward on 8 Trainium2 NeuronCores — one attention head per core.

Math (per head, all [4096] nodes):
    h    = x @ W                       [N, 128]
    ci   = x @ (W @ w_i)  (per-node)   [N]   (wv2 = [W@w_i, W@w_j] folded on host)
    cj   = x @ (W @ w_j)  (per-node)   [N]
    s[j, i]  = ci[i] + cj[j] + M[j, i]        (M = 0 / -1e9 additive bf16 mask)
    u        = max(0.2*s, s)                  (= leaky_relu; DVE/GPSIMD stt op)
    e[j, i]  = exp(u)                         (ACT, bf16; masked entries exp to 0)
    yT[f, i] = sum_j h[j, f] * e[j, i]        (PE matmul, e moving, h stationary)
    rs[i]    = sum_j e[j, i]                  (PE matmul vs bf16 ones column)
    rT[f, i] = (W_r^T x^T)[f, i]              (mapped residual, transposed layout)
    y[i, f]  = yT[f, i] / rs[i] + rT[f, i]    (division/transpose/bias on host)

Layout/scheduling notes:
  - Entire attention path is bf16: PE moving operand streams at 2.4 GHz (vs
    1.2 for f32r), LDWEIGHTS gets FWL, DVE elementwise ops run in 2x mode.
    PSUM accumulation stays fp32, outputs ship as fp32.
  - Scores computed transposed (j on partitions): mask tiles load in natural
    row order, cj[j] is the per-partition scalar of one fused DVE
    scalar_tensor_tensor (ciB + cj + M), leaky is one more stt
    (s*0.2 max s) split columnwise between DVE and GPSIMD, then one ACT Exp
    pass. ACT is the per-tile critical engine at ~2.0us.
  - h is produced directly in [j, f] layout (stationary = xT 128-col chunks,
    moving = W chunks): no hT buffer and no PE transposes.
  - i split in two 2048 halves so PSUM holds yT (4 banks) + rowsum (4 banks).
  - Outputs stay transposed ([f, i]); host divides by rowsum and transposes.
"""
import sys

sys.path.insert(0, "/opt/trn_rl_repo")
from contextlib import ExitStack

import numpy as np
import ml_dtypes

import concourse.bass as bass
import concourse.tile as tile
from concourse import bacc, mybir
from concourse.bass_utils import run_bass_kernel_spmd

dt = mybir.dt
F32, BF16 = dt.float32, dt.bfloat16
AF = mybir.ActivationFunctionType
OP = mybir.AluOpType

N = 4096
IN_F = 512
HF = 128
HEADS = 8
SLOPE = 0.2
MASK_NEG = -1.0e9
HALF = 2048
NJT = N // 128  # 32 j-tiles
NMC = IN_F // 128  # 4 contraction chunks over in-features

CA = 808  # columns whose cj-add+leaky run on ACT (Prelu with per-partition bias)
MASK_MULT = False  # additive -1e9 mask, CCE-added into u by the DMA engines

_prog = None


def build_program():
    nc = bacc.Bacc("TRN2", target_bir_lowering=False, debug=False)
    xT_d = nc.dram_tensor("xT", [IN_F, N], BF16, kind="ExternalInput").ap()
    mask_d = nc.dram_tensor("mask", [N, N], BF16, kind="ExternalInput").ap()
    W_d = nc.dram_tensor("W", [IN_F, HF], BF16, kind="ExternalInput").ap()
    Wr_d = nc.dram_tensor("Wr", [IN_F, HF], BF16, kind="ExternalInput").ap()
    wv2_d = nc.dram_tensor("wv2", [IN_F, 2], BF16, kind="ExternalInput").ap()
    eye_d = nc.dram_tensor("eye", [128, 128], F32, kind="ExternalInput").ap()
    yT_d = nc.dram_tensor("yT", [HF, N], F32, kind="ExternalOutput").ap()
    rs_d = nc.dram_tensor("rs", [1, N], F32, kind="ExternalOutput").ap()
    rT_d = nc.dram_tensor("rT", [HF, N], F32, kind="ExternalOutput").ap()

    with tile.TileContext(nc) as tc, ExitStack() as ctx:
        persist = ctx.enter_context(tc.tile_pool(name="persist", bufs=1))
        xs = persist.tile([128, NMC * N], BF16, tag="xs")  # xT chunk mc at cols mc*N
        W_sb = persist.tile([128, NMC * HF], BF16, tag="W")
        Wr_sb = persist.tile([128, NMC * HF], BF16, tag="Wr")
        wv2_sb = persist.tile([128, 2 * NMC], BF16, tag="wv2")
        eye_sb = persist.tile([128, 128], F32, tag="eye")
        ones_bf = persist.tile([128, 1], BF16, tag="ones")
        ciB = persist.tile([128, N], BF16, tag="ciB")  # ci[i] broadcast on partitions
        cjT = persist.tile([128, NJT], F32, tag="cjT")  # cj col per j-tile
        h_sb = persist.tile([128, N], BF16, tag="h")  # h[j, f], j-tile slices

        # xs loads split per half so the ci/cj matmuls can start after 2MB
        for hf in range(2):
            o = hf * HALF
            for mc in range(NMC):
                nc.sync.dma_start(
                    xs[:, mc * N + o : mc * N + o + HALF],
                    xT_d[mc * 128 : (mc + 1) * 128, o : o + HALF],
                )
        for mc in range(NMC):
            nc.sync.dma_start(
                W_sb[:, mc * HF : (mc + 1) * HF], W_d[mc * 128 : (mc + 1) * 128, :]
            )
            nc.sync.dma_start(
                Wr_sb[:, mc * HF : (mc + 1) * HF], Wr_d[mc * 128 : (mc + 1) * 128, :]
            )
            nc.sync.dma_start(
                wv2_sb[:, 2 * mc : 2 * mc + 2], wv2_d[mc * 128 : (mc + 1) * 128, :]
            )
        nc.sync.dma_start(eye_sb[:], eye_d)
        nc.vector.memset(ones_bf[:], 1.0)

        # HAM warmup: ~4.5us of dummy matmuls while xs streams in, so the
        # phase-1 matmuls run at 2.4GHz instead of the cold 1.2GHz default
        with ExitStack() as pw:
            psw = pw.enter_context(tc.tile_pool(name="psw", bufs=1, space="PSUM"))
            ps_w = psw.tile([128, 128], F32, tag="warm")
            for r in range(40):
                nc.tensor.matmul(
                    ps_w[:], eye_sb[:], eye_sb[:], start=(r == 0), stop=(r == 39)
                )

        # Phase-2 pools opened first so their SBUF is disjoint from any
        # phase-1 scoped buffers.
        ph2 = ctx.enter_context(tc.tile_pool(name="ph2", bufs=4))
        inpool = ctx.enter_context(tc.tile_pool(name="inpool", bufs=3))
        epool = ctx.enter_context(tc.tile_pool(name="epool", bufs=3))
        tpool = ctx.enter_context(tc.tile_pool(name="tpool", bufs=2))
        fin = ctx.enter_context(tc.tile_pool(name="fin", bufs=2))

        # ---------- Phase 1a: ci/cj = wv2^T @ xT ----------
        with ExitStack() as p1:
            rows = p1.enter_context(tc.tile_pool(name="rows", bufs=1))
            cc_sb = rows.tile([2, N], F32, tag="cc_sb")
            ci_bfrow = rows.tile([1, N], BF16, tag="ci_bfrow")
            cj_row = rows.tile([1, N], F32, tag="cj_row")
            ones_row = rows.tile([1, 128], BF16, tag="ones_row")
            nc.vector.memset(ones_row[:], 1.0)
            with ExitStack() as pa:
                psc = pa.enter_context(tc.tile_pool(name="psc", bufs=2, space="PSUM"))
                for hf in range(2):
                    o = hf * HALF
                    ps_cc = psc.tile([2, HALF], F32, tag="cc")
                    for ck in range(HALF // 512):
                        for mc in range(NMC):
                            nc.tensor.matmul(
                                ps_cc[0:2, ck * 512 : (ck + 1) * 512],
                                wv2_sb[:, 2 * mc : 2 * mc + 2],
                                xs[
                                    :,
                                    mc * N + o + ck * 512 : mc * N + o + (ck + 1) * 512,
                                ],
                                start=(mc == 0),
                                stop=(mc == NMC - 1),
                            )
                    nc.vector.tensor_copy(cc_sb[0:2, o : o + HALF], ps_cc[0:2, :])
                    nc.scalar.copy(ci_bfrow[0:1, o : o + HALF], cc_sb[0:1, o : o + HALF])
                    # engine reads must be partition-0-based; DMA reads row 1
                    nc.sync.dma_start(
                        cj_row[0:1, o : o + HALF], cc_sb[1:2, o : o + HALF]
                    )
            # broadcast ci along partitions on the PE: ciB = ones_col x ci_row
            with ExitStack() as pb:
                pscb = pb.enter_context(tc.tile_pool(name="pscb", bufs=2, space="PSUM"))
                for hf in range(2):
                    o = hf * HALF
                    ps_cb = pscb.tile([128, HALF], F32, tag="cb")
                    for ck in range(HALF // 512):
                        nc.tensor.matmul(
                            ps_cb[:, ck * 512 : (ck + 1) * 512],
                            ones_row[0:1, :],
                            ci_bfrow[0:1, o + ck * 512 : o + (ck + 1) * 512],
                            start=True,
                            stop=True,
                        )
                    nc.scalar.copy(ciB[:, o : o + HALF], ps_cb[:])
            with ExitStack() as pb:
                psct = pb.enter_context(tc.tile_pool(name="psct", bufs=1, space="PSUM"))
                ps_cjT = psct.tile([128, NJT], F32, tag="cjT_ps")
                for jt in range(NJT):
                    nc.tensor.transpose(
                        ps_cjT[:, jt : jt + 1],
                        cj_row[0:1, jt * 128 : (jt + 1) * 128],
                        eye_sb[0:1, 0:1],
                    )
                nc.vector.tensor_copy(cjT[:], ps_cjT[:])

        # ---------- Phase 1b: h[j, f] and residT[f, i] ----------
        with ExitStack() as p1:
            rstage = p1.enter_context(tc.tile_pool(name="rstage", bufs=2))
            psh = p1.enter_context(tc.tile_pool(name="psh", bufs=1, space="PSUM"))
            for hf in range(2):
                o = hf * HALF
                ps_h = psh.tile([128, HALF], F32, tag="h")
                for jb in range(HALF // 128):
                    for mc in range(NMC):
                        nc.tensor.matmul(
                            ps_h[:, jb * 128 : (jb + 1) * 128],
                            xs[:, mc * N + o + jb * 128 : mc * N + o + (jb + 1) * 128],
                            W_sb[:, mc * HF : (mc + 1) * HF],
                            start=(mc == 0),
                            stop=(mc == NMC - 1),
                        )
                nc.scalar.copy(h_sb[:, o : o + HALF], ps_h[:])
            for hf in range(2):
                o = hf * HALF
                ps_rT = psh.tile([128, HALF], F32, tag="rT")
                for ck in range(HALF // 512):
                    for mc in range(NMC):
                        nc.tensor.matmul(
                            ps_rT[:, ck * 512 : (ck + 1) * 512],
                            Wr_sb[:, mc * HF : (mc + 1) * HF],
                            xs[:, mc * N + o + ck * 512 : mc * N + o + (ck + 1) * 512],
                            start=(mc == 0),
                            stop=(mc == NMC - 1),
                        )
                rT_sb = rstage.tile([128, HALF], F32, tag="rT_sb")
                nc.scalar.copy(rT_sb[:], ps_rT[:])
                nc.sync.dma_start(rT_d[:, o : o + HALF], rT_sb[:])

        # ---------- Phase 2: attention ----------
        for half in range(2):
            i0 = half * HALF
            with ExitStack() as pmm_ctx:
                pmm = pmm_ctx.enter_context(
                    tc.tile_pool(name=f"pmm{half}", bufs=1, space="PSUM")
                )
                yT_ps = pmm.tile([128, HALF], F32, tag="yT")
                rs_ps = pmm.tile([1, HALF], F32, tag="rs")

                for jp in range(NJT // 2):
                    # two j-tiles share one s/e buffer: one Exp per pair
                    s_t = inpool.tile([128, 2 * HALF], BF16, tag="s")
                    for sub in range(2):
                        jt = 2 * jp + sub
                        off = sub * HALF
                        # zone A [0:CA]: ACT Prelu reads ciB, adds cj, leaky
                        nc.scalar.activation(
                            s_t[:, off : off + CA],
                            ciB[:, i0 : i0 + CA],
                            AF.Prelu,
                            bias=cjT[:, jt : jt + 1],
                            alpha=SLOPE,
                        )
                        # zone B [CA:]: DVE ciB+cj, then max(0.2*s, s)
                        nc.vector.tensor_scalar_add(
                            s_t[:, off + CA : off + HALF],
                            ciB[:, i0 + CA : i0 + HALF],
                            cjT[:, jt : jt + 1],
                        )
                        t_t = tpool.tile([128, HALF - CA], BF16, tag="t")
                        nc.vector.tensor_scalar_mul(
                            t_t[:], s_t[:, off + CA : off + HALF], SLOPE
                        )
                        nc.vector.tensor_tensor(
                            s_t[:, off + CA : off + HALF],
                            s_t[:, off + CA : off + HALF],
                            t_t[:],
                            op=OP.max,
                        )
                    # DMA engines add the -1e9 mask into u post-leaky (CCE accum);
                    # exp then zeroes masked entries, matching the reference order
                    for sub in range(2):
                        jt = 2 * jp + sub
                        off = sub * HALF
                        nc.gpsimd.dma_start(
                            s_t[:, off : off + HALF],
                            mask_d[jt * 128 : (jt + 1) * 128, i0 : i0 + HALF],
                            accum_op=OP.add,
                        )
                    e_t = epool.tile([128, 2 * HALF], BF16, tag="e")
                    nc.scalar.activation(e_t[:], s_t[:], AF.Exp)

                    for sub in range(2):
                        jt = 2 * jp + sub
                        off = sub * HALF
                        hr = h_sb[:, jt * 128 : (jt + 1) * 128]
                        for c in range(HALF // 512):
                            nc.tensor.matmul(
                                yT_ps[:, c * 512 : (c + 1) * 512],
                                hr,
                                e_t[:, off + c * 512 : off + (c + 1) * 512],
                                start=(jt == 0),
                                stop=(jt == NJT - 1),
                            )
                        for c in range(HALF // 512):
                            nc.tensor.matmul(
                                rs_ps[0:1, c * 512 : (c + 1) * 512],
                                ones_bf[:],
                                e_t[:, off + c * 512 : off + (c + 1) * 512],
                                start=(jt == 0),
                                stop=(jt == NJT - 1),
                            )

                # chunked evacuation: half 0 via idle GPSIMD (overlaps half 1's
                # compute), half 1 via scalar (free at the tail); DMA per chunk
                yT_sb = fin.tile([128, HALF], F32, tag="yT_sb")
                for c in range(HALF // 512):
                    sl = slice(c * 512, (c + 1) * 512)
                    if half == 0:
                        nc.vector.tensor_copy(yT_sb[:, sl], yT_ps[:, sl])
                    else:
                        nc.scalar.copy(yT_sb[:, sl], yT_ps[:, sl])
                    nc.sync.dma_start(yT_d[:, i0 + c * 512 : i0 + (c + 1) * 512],
                                      yT_sb[:, sl])
                rs_sb = fin.tile([1, HALF], F32, tag="rs_sb")
                nc.vector.tensor_copy(rs_sb[:], rs_ps[:])
            nc.sync.dma_start(rs_d[0:1, i0 : i0 + HALF], rs_sb[:])

    nc.compile()
    return nc


def _get_program():
    global _prog
    if _prog is None:
        _prog = build_program()
    return _prog


def _prepare_in_maps(x, graph, W, w_i, w_j, W_r):
    xT = np.ascontiguousarray(x.T).astype(ml_dtypes.bfloat16)
    if MASK_MULT:
        mask = (graph > 0).astype(ml_dtypes.bfloat16)
    else:
        mask = np.where(graph > 0, np.float32(0.0), np.float32(MASK_NEG)).astype(
            ml_dtypes.bfloat16
        )
    eye = np.eye(128, dtype=np.float32)
    in_maps = []
    for c in range(HEADS):
        Wc = W[c].astype(np.float32)
        wv2 = np.concatenate([Wc @ w_i[c], Wc @ w_j[c]], axis=1)  # [IN_F, 2]
        in_maps.append(
            {
                "xT": xT,
                "mask": mask,
                "W": np.ascontiguousarray(Wc).astype(ml_dtypes.bfloat16),
                "Wr": np.ascontiguousarray(W_r[:, c * HF : (c + 1) * HF]).astype(
                    ml_dtypes.bfloat16
                ),
                "wv2": np.ascontiguousarray(wv2).astype(ml_dtypes.bfloat16),
                "eye": eye,
            }
        )
    return in_maps


def run(inputs, trace=False, **kwargs):
    """Run the SPMD kernel; returns (y_full, BassKernelResults)."""
    x = np.asarray(inputs["x"], dtype=np.float32)
    graph = np.asarray(inputs["graph"])
    W = np.asarray(inputs["W"], dtype=np.float32)
    w_i = np.asarray(inputs["w_i"], dtype=np.float32)
    w_j = np.asarray(inputs["w_j"], dtype=np.float32)
    W_r = np.asarray(inputs["W_r"], dtype=np.float32)
    bias = np.asarray(inputs["bias"], dtype=np.float32)

    nc = _get_program()
    in_maps = _prepare_in_maps(x, graph, W, w_i, w_j, W_r)
    br = run_bass_kernel_spmd(
        nc, in_maps, core_ids=list(range(HEADS)), trace=trace, **kwargs
    )
    heads = []
    for c in range(HEADS):
        yT = br.results[c]["yT"]  # [HF, N] unnormalized attention output
        rs = br.results[c]["rs"][0]  # [N] softmax row sums
        rT = br.results[c]["rT"]  # [HF, N] mapped residual (transposed)
        heads.append((yT / rs[None, :] + rT).T)
    y = np.concatenate(heads, axis=1)
    y = y + bias[None, :]
    return y.astype(np.float32), br


def kernel(**inputs):
    y, _ = run(inputs)
    return y


# revision 29
# speedup vs baseline: 1.3822x; 1.3822x over previous
"""GAT forward on 8 Trainium2 NeuronCores — one attention head per core.

Math (per head, all [4096] nodes):
    h    = x @ W                       [N, 128]
    ci   = x @ (W @ w_i)  (per-node)   [N]   (wv2 = [W@w_i, W@w_j] folded on host)
    cj   = x @ (W @ w_j)  (per-node)   [N]
    s[j, i]  = ci[i] + cj[j] + M[j, i]        (M = 0 / -1e9 additive bf16 mask)
    u        = max(0.2*s, s)                  (= leaky_relu; DVE/GPSIMD stt op)
    e[j, i]  = exp(u)                         (ACT, bf16; masked entries exp to 0)
    yT[f, i] = sum_j h[j, f] * e[j, i]        (PE matmul, e moving, h stationary)
    rs[i]    = sum_j e[j, i]                  (PE matmul vs bf16 ones column)
    rT[f, i] = (W_r^T x^T)[f, i]              (mapped residual, transposed layout)
    y[i, f]  = yT[f, i] / rs[i] + rT[f, i]    (division/transpose/bias on host)

Layout/scheduling notes:
  - Entire attention path is bf16: PE moving operand streams at 2.4 GHz (vs
    1.2 for f32r), LDWEIGHTS gets FWL, DVE elementwise ops run in 2x mode.
    PSUM accumulation stays fp32, outputs ship as fp32.
  - Scores computed transposed (j on partitions): mask tiles load in natural
    row order, cj[j] is the per-partition scalar of one fused DVE
    scalar_tensor_tensor (ciB + cj + M), leaky is one more stt
    (s*0.2 max s) split columnwise between DVE and GPSIMD, then one ACT Exp
    pass. ACT is the per-tile critical engine at ~2.0us.
  - h is produced directly in [j, f] layout (stationary = xT 128-col chunks,
    moving = W chunks): no hT buffer and no PE transposes.
  - i split in two 2048 halves so PSUM holds yT (4 banks) + rowsum (4 banks).
  - Outputs stay transposed ([f, i]); host divides by rowsum and transposes.
"""
import sys

sys.path.insert(0, "/opt/trn_rl_repo")
from contextlib import ExitStack

import numpy as np
import ml_dtypes

import concourse.bass as bass
import concourse.tile as tile
from concourse import bacc, mybir
from concourse.bass_utils import run_bass_kernel_spmd

dt = mybir.dt
F32, BF16 = dt.float32, dt.bfloat16
AF = mybir.ActivationFunctionType
OP = mybir.AluOpType

N = 4096
IN_F = 512
HF = 128
HEADS = 8
SLOPE = 0.2
MASK_NEG = -1.0e9
HALF = 2048
NJT = N // 128  # 32 j-tiles
NMC = IN_F // 128  # 4 contraction chunks over in-features

CA = 1104  # columns whose cj-add+leaky run on ACT (Prelu with per-partition bias)
MASK_MULT = False  # additive -1e9 mask

_prog = None


def build_program():
    nc = bacc.Bacc("TRN2", target_bir_lowering=False, debug=False)
    xT_d = nc.dram_tensor("xT", [IN_F, N], BF16, kind="ExternalInput").ap()
    mask_d = nc.dram_tensor("mask", [N, N], BF16, kind="ExternalInput").ap()
    W_d = nc.dram_tensor("W", [IN_F, HF], BF16, kind="ExternalInput").ap()
    Wr_d = nc.dram_tensor("Wr", [IN_F, HF], BF16, kind="ExternalInput").ap()
    wv2_d = nc.dram_tensor("wv2", [IN_F, 2], BF16, kind="ExternalInput").ap()
    eye_d = nc.dram_tensor("eye", [128, 128], F32, kind="ExternalInput").ap()
    yT_d = nc.dram_tensor("yT", [HF, N], F32, kind="ExternalOutput").ap()
    rs_d = nc.dram_tensor("rs", [1, N], F32, kind="ExternalOutput").ap()
    rT_d = nc.dram_tensor("rT", [HF, N], F32, kind="ExternalOutput").ap()

    with tile.TileContext(nc) as tc, ExitStack() as ctx:
        persist = ctx.enter_context(tc.tile_pool(name="persist", bufs=1))
        xs = persist.tile([128, NMC * N], BF16, tag="xs")  # xT chunk mc at cols mc*N
        W_sb = persist.tile([128, NMC * HF], BF16, tag="W")
        Wr_sb = persist.tile([128, NMC * HF], BF16, tag="Wr")
        wv2_sb = persist.tile([128, 2 * NMC], BF16, tag="wv2")
        eye_sb = persist.tile([128, 128], F32, tag="eye")
        ones_bf = persist.tile([128, 1], BF16, tag="ones")
        ciB = persist.tile([128, N], BF16, tag="ciB")  # ci[i] broadcast on partitions
        cjT = persist.tile([128, NJT], F32, tag="cjT")  # cj col per j-tile
        h_sb = persist.tile([128, N], BF16, tag="h")  # h[j, f], j-tile slices

        # xs loads split per half so the ci/cj matmuls can start after 2MB
        for hf in range(2):
            o = hf * HALF
            for mc in range(NMC):
                nc.sync.dma_start(
                    xs[:, mc * N + o : mc * N + o + HALF],
                    xT_d[mc * 128 : (mc + 1) * 128, o : o + HALF],
                )
        for mc in range(NMC):
            nc.sync.dma_start(
                W_sb[:, mc * HF : (mc + 1) * HF], W_d[mc * 128 : (mc + 1) * 128, :]
            )
            nc.sync.dma_start(
                Wr_sb[:, mc * HF : (mc + 1) * HF], Wr_d[mc * 128 : (mc + 1) * 128, :]
            )
            nc.sync.dma_start(
                wv2_sb[:, 2 * mc : 2 * mc + 2], wv2_d[mc * 128 : (mc + 1) * 128, :]
            )
        nc.sync.dma_start(eye_sb[:], eye_d)
        nc.vector.memset(ones_bf[:], 1.0)

        # Phase-2 pools opened first so their SBUF is disjoint from any
        # phase-1 scoped buffers.
        ph2 = ctx.enter_context(tc.tile_pool(name="ph2", bufs=4))
        inpool = ctx.enter_context(tc.tile_pool(name="inpool", bufs=3))
        epool = ctx.enter_context(tc.tile_pool(name="epool", bufs=3))
        tpool = ctx.enter_context(tc.tile_pool(name="tpool", bufs=2))
        fin = ctx.enter_context(tc.tile_pool(name="fin", bufs=2))

        # ---------- Phase 1a: ci/cj = wv2^T @ xT ----------
        with ExitStack() as p1:
            rows = p1.enter_context(tc.tile_pool(name="rows", bufs=1))
            cc_sb = rows.tile([2, N], F32, tag="cc_sb")
            ci_bfrow = rows.tile([1, N], BF16, tag="ci_bfrow")
            cj_row = rows.tile([1, N], F32, tag="cj_row")
            ones_row = rows.tile([1, 128], BF16, tag="ones_row")
            nc.vector.memset(ones_row[:], 1.0)
            with ExitStack() as pa:
                psc = pa.enter_context(tc.tile_pool(name="psc", bufs=2, space="PSUM"))
                for hf in range(2):
                    o = hf * HALF
                    ps_cc = psc.tile([2, HALF], F32, tag="cc")
                    for ck in range(HALF // 512):
                        for mc in range(NMC):
                            nc.tensor.matmul(
                                ps_cc[0:2, ck * 512 : (ck + 1) * 512],
                                wv2_sb[:, 2 * mc : 2 * mc + 2],
                                xs[
                                    :,
                                    mc * N + o + ck * 512 : mc * N + o + (ck + 1) * 512,
                                ],
                                start=(mc == 0),
                                stop=(mc == NMC - 1),
                            )
                    nc.vector.tensor_copy(cc_sb[0:2, o : o + HALF], ps_cc[0:2, :])
                    nc.scalar.copy(ci_bfrow[0:1, o : o + HALF], cc_sb[0:1, o : o + HALF])
                    # engine reads must be partition-0-based; DMA reads row 1
                    nc.sync.dma_start(
                        cj_row[0:1, o : o + HALF], cc_sb[1:2, o : o + HALF]
                    )
            # broadcast ci along partitions on the PE: ciB = ones_col x ci_row
            with ExitStack() as pb:
                pscb = pb.enter_context(tc.tile_pool(name="pscb", bufs=2, space="PSUM"))
                for hf in range(2):
                    o = hf * HALF
                    ps_cb = pscb.tile([128, HALF], F32, tag="cb")
                    for ck in range(HALF // 512):
                        nc.tensor.matmul(
                            ps_cb[:, ck * 512 : (ck + 1) * 512],
                            ones_row[0:1, :],
                            ci_bfrow[0:1, o + ck * 512 : o + (ck + 1) * 512],
                            start=True,
                            stop=True,
                        )
                    nc.scalar.copy(ciB[:, o : o + HALF], ps_cb[:])
            with ExitStack() as pb:
                psct = pb.enter_context(tc.tile_pool(name="psct", bufs=1, space="PSUM"))
                ps_cjT = psct.tile([128, NJT], F32, tag="cjT_ps")
                for jt in range(NJT):
                    nc.tensor.transpose(
                        ps_cjT[:, jt : jt + 1],
                        cj_row[0:1, jt * 128 : (jt + 1) * 128],
                        eye_sb[0:1, 0:1],
                    )
                nc.vector.tensor_copy(cjT[:], ps_cjT[:])

        # ---------- Phase 1b: h[j, f] and residT[f, i] ----------
        with ExitStack() as p1:
            rstage = p1.enter_context(tc.tile_pool(name="rstage", bufs=2))
            psh = p1.enter_context(tc.tile_pool(name="psh", bufs=1, space="PSUM"))
            for hf in range(2):
                o = hf * HALF
                ps_h = psh.tile([128, HALF], F32, tag="h")
                for jb in range(HALF // 128):
                    for mc in range(NMC):
                        nc.tensor.matmul(
                            ps_h[:, jb * 128 : (jb + 1) * 128],
                            xs[:, mc * N + o + jb * 128 : mc * N + o + (jb + 1) * 128],
                            W_sb[:, mc * HF : (mc + 1) * HF],
                            start=(mc == 0),
                            stop=(mc == NMC - 1),
                        )
                nc.scalar.copy(h_sb[:, o : o + HALF], ps_h[:])
            for hf in range(2):
                o = hf * HALF
                ps_rT = psh.tile([128, HALF], F32, tag="rT")
                for ck in range(HALF // 512):
                    for mc in range(NMC):
                        nc.tensor.matmul(
                            ps_rT[:, ck * 512 : (ck + 1) * 512],
                            Wr_sb[:, mc * HF : (mc + 1) * HF],
                            xs[:, mc * N + o + ck * 512 : mc * N + o + (ck + 1) * 512],
                            start=(mc == 0),
                            stop=(mc == NMC - 1),
                        )
                rT_sb = rstage.tile([128, HALF], F32, tag="rT_sb")
                nc.scalar.copy(rT_sb[:], ps_rT[:])
                nc.sync.dma_start(rT_d[:, o : o + HALF], rT_sb[:])

        # ---------- Phase 2: attention ----------
        for half in range(2):
            i0 = half * HALF
            with ExitStack() as pmm_ctx:
                pmm = pmm_ctx.enter_context(
                    tc.tile_pool(name=f"pmm{half}", bufs=1, space="PSUM")
                )
                yT_ps = pmm.tile([128, HALF], F32, tag="yT")
                rs_ps = pmm.tile([1, HALF], F32, tag="rs")

                for jp in range(NJT // 2):
                    # two j-tiles share one s/e buffer: one Exp per pair
                    s_t = inpool.tile([128, 2 * HALF], BF16, tag="s")
                    for sub in range(2):
                        jt = 2 * jp + sub
                        off = sub * HALF
                        m_t = ph2.tile([128, HALF], BF16, tag="m")
                        nc.sync.dma_start(
                            m_t[:], mask_d[jt * 128 : (jt + 1) * 128, i0 : i0 + HALF]
                        )
                        # zone A [0:CA]: DVE adds mask, ACT Prelu adds cj + leaky
                        nc.vector.tensor_tensor(
                            s_t[:, off : off + CA],
                            ciB[:, i0 : i0 + CA],
                            m_t[:, 0:CA],
                            op=OP.add,
                        )
                        nc.scalar.activation(
                            s_t[:, off : off + CA],
                            s_t[:, off : off + CA],
                            AF.Prelu,
                            bias=cjT[:, jt : jt + 1],
                            alpha=SLOPE,
                        )
                        # zone B [CA:]: DVE 3-operand add, then max(0.2*s, s)
                        nc.vector.scalar_tensor_tensor(
                            s_t[:, off + CA : off + HALF],
                            m_t[:, CA:HALF],
                            cjT[:, jt : jt + 1],
                            ciB[:, i0 + CA : i0 + HALF],
                            op0=OP.add,
                            op1=OP.add,
                        )
                        t_t = tpool.tile([128, HALF - CA], BF16, tag="t")
                        nc.vector.tensor_scalar_mul(
                            t_t[:], s_t[:, off + CA : off + HALF], SLOPE
                        )
                        nc.vector.tensor_tensor(
                            s_t[:, off + CA : off + HALF],
                            s_t[:, off + CA : off + HALF],
                            t_t[:],
                            op=OP.max,
                        )
                    e_t = epool.tile([128, 2 * HALF], BF16, tag="e")
                    nc.scalar.activation(e_t[:], s_t[:], AF.Exp)

                    for sub in range(2):
                        jt = 2 * jp + sub
                        off = sub * HALF
                        hr = h_sb[:, jt * 128 : (jt + 1) * 128]
                        for c in range(HALF // 512):
                            nc.tensor.matmul(
                                yT_ps[:, c * 512 : (c + 1) * 512],
                                hr,
                                e_t[:, off + c * 512 : off + (c + 1) * 512],
                                start=(jt == 0),
                                stop=(jt == NJT - 1),
                            )
                        for c in range(HALF // 512):
                            nc.tensor.matmul(
                                rs_ps[0:1, c * 512 : (c + 1) * 512],
                                ones_bf[:],
                                e_t[:, off + c * 512 : off + (c + 1) * 512],
                                start=(jt == 0),
                                stop=(jt == NJT - 1),
                            )

                # chunked evacuation: half 0 via idle GPSIMD (overlaps half 1's
                # compute), half 1 via scalar (free at the tail); DMA per chunk
                yT_sb = fin.tile([128, HALF], F32, tag="yT_sb")
                for c in range(HALF // 512):
                    sl = slice(c * 512, (c + 1) * 512)
                    if half == 0:
                        nc.vector.tensor_copy(yT_sb[:, sl], yT_ps[:, sl])
                    else:
                        nc.scalar.copy(yT_sb[:, sl], yT_ps[:, sl])
                    nc.sync.dma_start(yT_d[:, i0 + c * 512 : i0 + (c + 1) * 512],
                                      yT_sb[:, sl])
                rs_sb = fin.tile([1, HALF], F32, tag="rs_sb")
                nc.vector.tensor_copy(rs_sb[:], rs_ps[:])
            nc.sync.dma_start(rs_d[0:1, i0 : i0 + HALF], rs_sb[:])

    nc.compile()
    return nc


def _get_program():
    global _prog
    if _prog is None:
        _prog = build_program()
    return _prog


def _prepare_in_maps(x, graph, W, w_i, w_j, W_r):
    xT = np.ascontiguousarray(x.T).astype(ml_dtypes.bfloat16)
    if MASK_MULT:
        mask = (graph > 0).astype(ml_dtypes.bfloat16)
    else:
        mask = np.where(graph > 0, np.float32(0.0), np.float32(MASK_NEG)).astype(
            ml_dtypes.bfloat16
        )
    eye = np.eye(128, dtype=np.float32)
    in_maps = []
    for c in range(HEADS):
        Wc = W[c].astype(np.float32)
        wv2 = np.concatenate([Wc @ w_i[c], Wc @ w_j[c]], axis=1)  # [IN_F, 2]
        in_maps.append(
            {
                "xT": xT,
                "mask": mask,
                "W": np.ascontiguousarray(Wc).astype(ml_dtypes.bfloat16),
                "Wr": np.ascontiguousarray(W_r[:, c * HF : (c + 1) * HF]).astype(
                    ml_dtypes.bfloat16
                ),
                "wv2": np.ascontiguousarray(wv2).astype(ml_dtypes.bfloat16),
                "eye": eye,
            }
        )
    return in_maps


def run(inputs, trace=False, **kwargs):
    """Run the SPMD kernel; returns (y_full, BassKernelResults)."""
    x = np.asarray(inputs["x"], dtype=np.float32)
    graph = np.asarray(inputs["graph"])
    W = np.asarray(inputs["W"], dtype=np.float32)
    w_i = np.asarray(inputs["w_i"], dtype=np.float32)
    w_j = np.asarray(inputs["w_j"], dtype=np.float32)
    W_r = np.asarray(inputs["W_r"], dtype=np.float32)
    bias = np.asarray(inputs["bias"], dtype=np.float32)

    nc = _get_program()
    in_maps = _prepare_in_maps(x, graph, W, w_i, w_j, W_r)
    br = run_bass_kernel_spmd(
        nc, in_maps, core_ids=list(range(HEADS)), trace=trace, **kwargs
    )
    heads = []
    for c in range(HEADS):
        yT = br.results[c]["yT"]  # [HF, N] unnormalized attention output
        rs = br.results[c]["rs"][0]  # [N] softmax row sums
        rT = br.results[c]["rT"]  # [HF, N] mapped residual (transposed)
        heads.append((yT / rs[None, :] + rT).T)
    y = np.concatenate(heads, axis=1)
    y = y + bias[None, :]
    return y.astype(np.float32), br


def kernel(**inputs):
    y, _ = run(inputs)
    return y


# revision 30
# speedup vs baseline: 1.4072x; 1.0180x over previous
"""GAT forward on 8 Trainium2 NeuronCores — one attention head per core.

Math (per head, all [4096] nodes):
    h    = x @ W                       [N, 128]
    ci   = x @ (W @ w_i)  (per-node)   [N]   (wv2 = [W@w_i, W@w_j] folded on host)
    cj   = x @ (W @ w_j)  (per-node)   [N]
    s[j, i]  = ci[i] + cj[j] + M[j, i]        (M = 0 / -1e9 additive bf16 mask)
    u        = max(0.2*s, s)                  (= leaky_relu; DVE/GPSIMD stt op)
    e[j, i]  = exp(u)                         (ACT, bf16; masked entries exp to 0)
    yT[f, i] = sum_j h[j, f] * e[j, i]        (PE matmul, e moving, h stationary)
    rs[i]    = sum_j e[j, i]                  (PE matmul vs bf16 ones column)
    rT[f, i] = (W_r^T x^T)[f, i]              (mapped residual, transposed layout)
    y[i, f]  = yT[f, i] / rs[i] + rT[f, i]    (division/transpose/bias on host)

Layout/scheduling notes:
  - Entire attention path is bf16: PE moving operand streams at 2.4 GHz (vs
    1.2 for f32r), LDWEIGHTS gets FWL, DVE elementwise ops run in 2x mode.
    PSUM accumulation stays fp32, outputs ship as fp32.
  - Scores computed transposed (j on partitions): mask tiles load in natural
    row order, cj[j] is the per-partition scalar of one fused DVE
    scalar_tensor_tensor (ciB + cj + M), leaky is one more stt
    (s*0.2 max s) split columnwise between DVE and GPSIMD, then one ACT Exp
    pass. ACT is the per-tile critical engine at ~2.0us.
  - h is produced directly in [j, f] layout (stationary = xT 128-col chunks,
    moving = W chunks): no hT buffer and no PE transposes.
  - i split in two 2048 halves so PSUM holds yT (4 banks) + rowsum (4 banks).
  - Outputs stay transposed ([f, i]); host divides by rowsum and transposes.
"""
import sys

sys.path.insert(0, "/opt/trn_rl_repo")
from contextlib import ExitStack

import numpy as np
import ml_dtypes

import concourse.bass as bass
import concourse.tile as tile
from concourse import bacc, mybir
from concourse.bass_utils import run_bass_kernel_spmd

dt = mybir.dt
F32, BF16 = dt.float32, dt.bfloat16
AF = mybir.ActivationFunctionType
OP = mybir.AluOpType

N = 4096
IN_F = 512
HF = 128
HEADS = 8
SLOPE = 0.2
MASK_NEG = -1.0e9
HALF = 2048
NJT = N // 128  # 32 j-tiles
NMC = IN_F // 128  # 4 contraction chunks over in-features

CA = 1104  # columns whose cj-add+leaky run on ACT (Prelu with per-partition bias)
MASK_MULT = False  # additive -1e9 mask

_prog = None


def build_program():
    nc = bacc.Bacc("TRN2", target_bir_lowering=False, debug=False)
    xT_d = nc.dram_tensor("xT", [IN_F, N], BF16, kind="ExternalInput").ap()
    mask_d = nc.dram_tensor("mask", [N, N], BF16, kind="ExternalInput").ap()
    W_d = nc.dram_tensor("W", [IN_F, HF], BF16, kind="ExternalInput").ap()
    Wr_d = nc.dram_tensor("Wr", [IN_F, HF], BF16, kind="ExternalInput").ap()
    wv2_d = nc.dram_tensor("wv2", [IN_F, 2], BF16, kind="ExternalInput").ap()
    eye_d = nc.dram_tensor("eye", [128, 128], F32, kind="ExternalInput").ap()
    yT_d = nc.dram_tensor("yT", [HF, N], F32, kind="ExternalOutput").ap()
    rs_d = nc.dram_tensor("rs", [1, N], F32, kind="ExternalOutput").ap()
    rT_d = nc.dram_tensor("rT", [HF, N], F32, kind="ExternalOutput").ap()

    with tile.TileContext(nc) as tc, ExitStack() as ctx:
        persist = ctx.enter_context(tc.tile_pool(name="persist", bufs=1))
        xs = persist.tile([128, NMC * N], BF16, tag="xs")  # xT chunk mc at cols mc*N
        W_sb = persist.tile([128, NMC * HF], BF16, tag="W")
        Wr_sb = persist.tile([128, NMC * HF], BF16, tag="Wr")
        wv2_sb = persist.tile([128, 2 * NMC], BF16, tag="wv2")
        eye_sb = persist.tile([128, 128], F32, tag="eye")
        ones_bf = persist.tile([128, 1], BF16, tag="ones")
        ciB = persist.tile([128, N], BF16, tag="ciB")  # ci[i] broadcast on partitions
        cjT = persist.tile([128, NJT], F32, tag="cjT")  # cj col per j-tile
        h_sb = persist.tile([128, N], BF16, tag="h")  # h[j, f], j-tile slices

        # xs loads split per half so the ci/cj matmuls can start after 2MB
        for hf in range(2):
            o = hf * HALF
            for mc in range(NMC):
                nc.sync.dma_start(
                    xs[:, mc * N + o : mc * N + o + HALF],
                    xT_d[mc * 128 : (mc + 1) * 128, o : o + HALF],
                )
        for mc in range(NMC):
            nc.sync.dma_start(
                W_sb[:, mc * HF : (mc + 1) * HF], W_d[mc * 128 : (mc + 1) * 128, :]
            )
            nc.sync.dma_start(
                Wr_sb[:, mc * HF : (mc + 1) * HF], Wr_d[mc * 128 : (mc + 1) * 128, :]
            )
            nc.sync.dma_start(
                wv2_sb[:, 2 * mc : 2 * mc + 2], wv2_d[mc * 128 : (mc + 1) * 128, :]
            )
        nc.sync.dma_start(eye_sb[:], eye_d)
        nc.vector.memset(ones_bf[:], 1.0)

        # Phase-2 pools opened first so their SBUF is disjoint from any
        # phase-1 scoped buffers.
        ph2 = ctx.enter_context(tc.tile_pool(name="ph2", bufs=4))
        inpool = ctx.enter_context(tc.tile_pool(name="inpool", bufs=3))
        epool = ctx.enter_context(tc.tile_pool(name="epool", bufs=3))
        tpool = ctx.enter_context(tc.tile_pool(name="tpool", bufs=2))
        fin = ctx.enter_context(tc.tile_pool(name="fin", bufs=2))

        # ---------- Phase 1: ci/cj + h interleaved, then residT ----------
        # cicj groups interleave with h groups so the PE stays dense (warms
        # fast), and each 512-col ci chunk flows evac -> row -> broadcast ->
        # ciB while the PE runs the next groups.
        with ExitStack() as p1:
            rows = p1.enter_context(tc.tile_pool(name="rows", bufs=1))
            cc_sb = rows.tile([2, N], F32, tag="cc_sb")
            ci_bfrow = rows.tile([1, N], BF16, tag="ci_bfrow")
            cj_row = rows.tile([1, N], F32, tag="cj_row")
            ones_row = rows.tile([1, 128], BF16, tag="ones_row")
            nc.vector.memset(ones_row[:], 1.0)
            with ExitStack() as ph1:
                psh = ph1.enter_context(tc.tile_pool(name="psh", bufs=1, space="PSUM"))
                with ExitStack() as pa:
                    psc = pa.enter_context(
                        tc.tile_pool(name="psc", bufs=2, space="PSUM")
                    )
                    pscb = pa.enter_context(
                        tc.tile_pool(name="pscb", bufs=2, space="PSUM")
                    )
                    ps_h = None
                    for ck in range(N // 512):
                        hf = ck // 4
                        o = hf * HALF
                        gs = ck * 512
                        if ck % 4 == 0:
                            if ps_h is not None:
                                nc.vector.tensor_copy(
                                    h_sb[:, (hf - 1) * HALF : hf * HALF], ps_h[:]
                                )
                            ps_h = psh.tile([128, HALF], F32, tag="h")
                        ps_cc = psc.tile([2, 512], F32, tag="cc")
                        for mc in range(NMC):
                            nc.tensor.matmul(
                                ps_cc[0:2, :],
                                wv2_sb[:, 2 * mc : 2 * mc + 2],
                                xs[:, mc * N + gs : mc * N + gs + 512],
                                start=(mc == 0),
                                stop=(mc == NMC - 1),
                            )
                        for t in range(4):
                            jb = ck * 4 + t
                            lo = (jb % 16) * 128
                            for mc in range(NMC):
                                nc.tensor.matmul(
                                    ps_h[:, lo : lo + 128],
                                    xs[:, mc * N + jb * 128 : mc * N + (jb + 1) * 128],
                                    W_sb[:, mc * HF : (mc + 1) * HF],
                                    start=(mc == 0),
                                    stop=(mc == NMC - 1),
                                )
                        nc.vector.tensor_copy(cc_sb[0:2, gs : gs + 512], ps_cc[0:2, :])
                        nc.scalar.copy(
                            ci_bfrow[0:1, gs : gs + 512], cc_sb[0:1, gs : gs + 512]
                        )
                        # engine reads must be partition-0-based; DMA reads row 1
                        nc.sync.dma_start(
                            cj_row[0:1, gs : gs + 512], cc_sb[1:2, gs : gs + 512]
                        )
                        ps_cb = pscb.tile([128, 512], F32, tag="cb")
                        nc.tensor.matmul(
                            ps_cb[:],
                            ones_row[0:1, :],
                            ci_bfrow[0:1, gs : gs + 512],
                            start=True,
                            stop=True,
                        )
                        nc.scalar.copy(ciB[:, gs : gs + 512], ps_cb[:])
                    nc.vector.tensor_copy(h_sb[:, HALF : 2 * HALF], ps_h[:])
                with ExitStack() as pb:
                    psct = pb.enter_context(
                        tc.tile_pool(name="psct", bufs=1, space="PSUM")
                    )
                    ps_cjT = psct.tile([128, NJT], F32, tag="cjT_ps")
                    for jt in range(NJT):
                        nc.tensor.transpose(
                            ps_cjT[:, jt : jt + 1],
                            cj_row[0:1, jt * 128 : (jt + 1) * 128],
                            eye_sb[0:1, 0:1],
                        )
                    nc.vector.tensor_copy(cjT[:], ps_cjT[:])

        # residT after h (PSUM banks reused)
        with ExitStack() as p1:
            rstage = p1.enter_context(tc.tile_pool(name="rstage", bufs=2))
            psr = p1.enter_context(tc.tile_pool(name="psr", bufs=1, space="PSUM"))
            for hf in range(2):
                o = hf * HALF
                ps_rT = psr.tile([128, HALF], F32, tag="rT")
                for ck in range(HALF // 512):
                    for mc in range(NMC):
                        nc.tensor.matmul(
                            ps_rT[:, ck * 512 : (ck + 1) * 512],
                            Wr_sb[:, mc * HF : (mc + 1) * HF],
                            xs[:, mc * N + o + ck * 512 : mc * N + o + (ck + 1) * 512],
                            start=(mc == 0),
                            stop=(mc == NMC - 1),
                        )
                rT_sb = rstage.tile([128, HALF], F32, tag="rT_sb")
                nc.scalar.copy(rT_sb[:], ps_rT[:])
                nc.sync.dma_start(rT_d[:, o : o + HALF], rT_sb[:])

        # ---------- Phase 2: attention ----------
        for half in range(2):
            i0 = half * HALF
            with ExitStack() as pmm_ctx:
                pmm = pmm_ctx.enter_context(
                    tc.tile_pool(name=f"pmm{half}", bufs=1, space="PSUM")
                )
                yT_ps = pmm.tile([128, HALF], F32, tag="yT")
                rs_ps = pmm.tile([1, HALF], F32, tag="rs")

                for jp in range(NJT // 2):
                    # two j-tiles share one s/e buffer: one Exp per pair
                    s_t = inpool.tile([128, 2 * HALF], BF16, tag="s")
                    for sub in range(2):
                        jt = 2 * jp + sub
                        off = sub * HALF
                        m_t = ph2.tile([128, HALF], BF16, tag="m")
                        nc.sync.dma_start(
                            m_t[:], mask_d[jt * 128 : (jt + 1) * 128, i0 : i0 + HALF]
                        )
                        # zone A [0:CA]: DVE adds mask, ACT Prelu adds cj + leaky
                        nc.vector.tensor_tensor(
                            s_t[:, off : off + CA],
                            ciB[:, i0 : i0 + CA],
                            m_t[:, 0:CA],
                            op=OP.add,
                        )
                        nc.scalar.activation(
                            s_t[:, off : off + CA],
                            s_t[:, off : off + CA],
                            AF.Prelu,
                            bias=cjT[:, jt : jt + 1],
                            alpha=SLOPE,
                        )
                        # zone B [CA:]: DVE 3-operand add, then max(0.2*s, s)
                        nc.vector.scalar_tensor_tensor(
                            s_t[:, off + CA : off + HALF],
                            m_t[:, CA:HALF],
                            cjT[:, jt : jt + 1],
                            ciB[:, i0 + CA : i0 + HALF],
                            op0=OP.add,
                            op1=OP.add,
                        )
                        t_t = tpool.tile([128, HALF - CA], BF16, tag="t")
                        nc.vector.tensor_scalar_mul(
                            t_t[:], s_t[:, off + CA : off + HALF], SLOPE
                        )
                        nc.vector.tensor_tensor(
                            s_t[:, off + CA : off + HALF],
                            s_t[:, off + CA : off + HALF],
                            t_t[:],
                            op=OP.max,
                        )
                    e_t = epool.tile([128, 2 * HALF], BF16, tag="e")
                    nc.scalar.activation(e_t[:], s_t[:], AF.Exp)

                    for sub in range(2):
                        jt = 2 * jp + sub
                        off = sub * HALF
                        hr = h_sb[:, jt * 128 : (jt + 1) * 128]
                        for c in range(HALF // 512):
                            nc.tensor.matmul(
                                yT_ps[:, c * 512 : (c + 1) * 512],
                                hr,
                                e_t[:, off + c * 512 : off + (c + 1) * 512],
                                start=(jt == 0),
                                stop=(jt == NJT - 1),
                            )
                        for c in range(HALF // 512):
                            nc.tensor.matmul(
                                rs_ps[0:1, c * 512 : (c + 1) * 512],
                                ones_bf[:],
                                e_t[:, off + c * 512 : off + (c + 1) * 512],
                                start=(jt == 0),
                                stop=(jt == NJT - 1),
                            )

                # chunked evacuation: half 0 via idle GPSIMD (overlaps half 1's
                # compute), half 1 via scalar (free at the tail); DMA per chunk
                yT_sb = fin.tile([128, HALF], F32, tag="yT_sb")
                for c in range(HALF // 512):
                    sl = slice(c * 512, (c + 1) * 512)
                    if half == 0:
                        nc.vector.tensor_copy(yT_sb[:, sl], yT_ps[:, sl])
                    else:
                        nc.scalar.copy(yT_sb[:, sl], yT_ps[:, sl])
                    nc.sync.dma_start(yT_d[:, i0 + c * 512 : i0 + (c + 1) * 512],
                                      yT_sb[:, sl])
                rs_sb = fin.tile([1, HALF], F32, tag="rs_sb")
                nc.vector.tensor_copy(rs_sb[:], rs_ps[:])
            nc.sync.dma_start(rs_d[0:1, i0 : i0 + HALF], rs_sb[:])

    nc.compile()
    return nc


def _get_program():
    global _prog
    if _prog is None:
        _prog = build_program()
    return _prog


def _prepare_in_maps(x, graph, W, w_i, w_j, W_r):
    xT = np.ascontiguousarray(x.T).astype(ml_dtypes.bfloat16)
    if MASK_MULT:
        mask = (graph > 0).astype(ml_dtypes.bfloat16)
    else:
        mask = np.where(graph > 0, np.float32(0.0), np.float32(MASK_NEG)).astype(
            ml_dtypes.bfloat16
        )
    eye = np.eye(128, dtype=np.float32)
    in_maps = []
    for c in range(HEADS):
        Wc = W[c].astype(np.float32)
        wv2 = np.concatenate([Wc @ w_i[c], Wc @ w_j[c]], axis=1)  # [IN_F, 2]
        in_maps.append(
            {
                "xT": xT,
                "mask": mask,
                "W": np.ascontiguousarray(Wc).astype(ml_dtypes.bfloat16),
                "Wr": np.ascontiguousarray(W_r[:, c * HF : (c + 1) * HF]).astype(
                    ml_dtypes.bfloat16
                ),
                "wv2": np.ascontiguousarray(wv2).astype(ml_dtypes.bfloat16),
                "eye": eye,
            }
        )
    return in_maps


def run(inputs, trace=False, **kwargs):
    """Run the SPMD kernel; returns (y_full, BassKernelResults)."""
    x = np.asarray(inputs["x"], dtype=np.float32)
    graph = np.asarray(inputs["graph"])
    W = np.asarray(inputs["W"], dtype=np.float32)
    w_i = np.asarray(inputs["w_i"], dtype=np.float32)
    w_j = np.asarray(inputs["w_j"], dtype=np.float32)
    W_r = np.asarray(inputs["W_r"], dtype=np.float32)
    bias = np.asarray(inputs["bias"], dtype=np.float32)

    nc = _get_program()
    in_maps = _prepare_in_maps(x, graph, W, w_i, w_j, W_r)
    br = run_bass_kernel_spmd(
        nc, in_maps, core_ids=list(range(HEADS)), trace=trace, **kwargs
    )
    heads = []
    for c in range(HEADS):
        yT = br.results[c]["yT"]  # [HF, N] unnormalized attention output
        rs = br.results[c]["rs"][0]  # [N] softmax row sums
        rT = br.results[c]["rT"]  # [HF, N] mapped residual (transposed)
        heads.append((yT / rs[None, :] + rT).T)
    y = np.concatenate(heads, axis=1)
    y = y + bias[None, :]
    return y.astype(np.float32), br


def kernel(**inputs):
    y, _ = run(inputs)
    return y
